# revision 35
# baseline (speedup 1.0000x reference)
"""DepTreeLSTM forward on 8 Trainium2 NeuronCores (Bass/Tile) — v3.

Forest of T=4096 full binary trees (depth 5, 63 nodes each), TreeLSTM swept
level-by-level (leaves -> root). Trees are data-parallel: 512 trees per core.

Channels-on-partitions layout: per-node work is column-parallel; within a
core, node columns are ordered (block, level, tree, pos) so the children of
parent column m at level d are columns 2m, 2m+1 of level d-1 (stride-2
access, no indirection).

Structure:
  - emb-side matmuls run as fp8(e4m3) hi/lo DoubleRow pairs: the host splits
    emb and the W weights into fp8 hi + lo planes; W@emb = Whi@xhi + Whi@xlo
    + Wlo@xhi, each a 256-deep DoubleRow matmul.  Weights are pre-scaled by
    16 (fp8 normal range); the 1/16 dequant is folded into the activation
    `scale`.  h-side (U) matmuls stay bf16 (also pre-scaled by 16).
  - all biases in this problem are zero, so sigmoid over [i|o|u'] is ONE
    activation instruction (u' = 2u via doubled u-weights; tanh(u) =
    2*sig(2u)-1 is fixed up in the c assembly), and sigmoid over [f0|f1] is
    one more.  tanh(c) is the third.
  - ty rows are pre-replicated to 128 partitions in DRAM (plain HWDGE load);
    h is stored to DRAM as bf16, one DMA per 2048-col sub-slab.
  - per level: ht1/ct1 = pairsum(ty*h / ty*c), hsum/csum = pairsum(h / c);
    c = si*(2*su-1) + s0*csum + (s1-s0)*ct1; h = so*tanh(c).
  - chunks flow through an explicit 3-stage software pipeline (A: child
    reductions + emb matmuls; B: h-side matmuls, sigmoid acts, c assembly;
    C: tanh, h multiply, store) so every engine's in-order sequencer always
    has ready work from an older chunk ahead of it.
"""

import math

import numpy as np
import ml_dtypes

import concourse.bass as bass
import concourse.tile as tile
from concourse import mybir
from concourse.bass_utils import run_bass_kernel_spmd

BF16 = ml_dtypes.bfloat16
FP8 = ml_dtypes.float8_e4m3
F32 = np.float32

# ---------------- problem constants (hardcoded) ----------------
T, C, D, E, H = 4096, 2, 5, 256, 128
COUNTS = [C ** (D - d) for d in range(D + 1)]      # [32,16,8,4,2,1]
OFFS = [0, 32, 48, 56, 60, 62]
S = 63
N = T * S
NCORES = 8
TPC = T // NCORES                                   # 512 trees / core
NBLK = 4                                            # tree blocks / core
BT = TPC // NBLK                                    # 128 trees / block
LVL_M = [BT * c for c in COUNTS]                    # [8192,4096,2048,1024,512,256]
BLK_COLS = BT * S                                   # 16128
CORE_COLS = TPC * S                                 # 32256
TY_BLK = 2 * sum(LVL_M[1:])                         # 15872 child slots / block
TY_TOTAL = NBLK * TY_BLK
MC = 512                                            # chunk (= PSUM bank)
SUB = 2048                                          # emb/ty staging sub-slab
WSC = 16.0                                          # fp8 weight pre-scale

SIG = mybir.ActivationFunctionType.Sigmoid
TANH = mybir.ActivationFunctionType.Tanh
MUL = mybir.AluOpType.mult
ADD = mybir.AluOpType.add
SUBT = mybir.AluOpType.subtract
DR = mybir.MatmulPerfMode.DoubleRow

LAST_EXEC_NS = None
TRACE_TAGS = None  # set to a list to record (label, id_lo, id_hi) per stage


def split_waits(nc, nop_max=1, keep_max=1):
    """Walrus in this container rejects instructions with too many sem-waits
    (Drain: 0 allowed, NoOp: 1, others: 2). Move excess waits onto inserted
    NoOps, one wait each."""
    n_fix = 0
    for f in nc.m.functions:
        for bb in f.blocks:
            insts = bb.instructions
            i = 0
            while i < len(insts):
                ins = insts[i]
                si = getattr(ins, "sync_info", None)
                ow = list(si.on_wait) if si and si.on_wait else []
                keep = 0 if type(ins).__name__ == "InstDrain" else keep_max
                if len(ow) > keep:
                    extra = ow[:len(ow) - keep]
                    si.on_wait = ow[len(ow) - keep:]
                    k = 0
                    while extra:
                        chunk, extra = extra[:nop_max], extra[nop_max:]
                        nop = mybir.InstNoOp(
                            name=f"I-wsplit-{ins.name}-{k}", engine=ins.engine,
                            ins=[], outs=[])
                        nop.sync_info = type(si)(on_wait=chunk, on_update=[])
                        insts.insert(i, nop)
                        i += 1
                        k += 1
                        n_fix += 1
                i += 1
    return n_fix


def _lvl_off(blk, d):
    return blk * BLK_COLS + sum(LVL_M[:d])


def _ty_off(blk, d):
    return blk * TY_BLK + 2 * sum(LVL_M[1:d])


# wf8 slots [128, 8, 2, 128]: per gate g in (i,o,u',f): slot 2g = (W0hi,W1hi),
# slot 2g+1 = (W0lo,W1lo).  u' slots hold 2*W_u.
# wb slots [128, 10, 128]: U0_i, U1d_i, U0_o, U1d_o, U0_u', U1d_u', A, Ad, B, Bd
# (U1d = U1 - U0; u' slots hold 2*U_u; all * WSC)


def _interleave(a, b):
    out = []
    ia = ib = 0
    while ia < len(a) or ib < len(b):
        if ia < len(a):
            out.append(a[ia])
            ia += 1
        if ib < len(b):
            out.append(b[ib])
            ib += 1
    return out


def _chunk_stream(mode, sub):
    """Ordered list of (blk, d, m0, sw, c0, mc) chunk descriptors."""
    def level(blk, d):
        out = []
        M = LVL_M[d]
        for m0 in range(0, M, sub):
            sw = min(sub, M - m0)
            for c0 in range(m0, m0 + sw, MC):
                out.append((blk, d, m0, sw, c0, min(MC, m0 + sw - c0)))
        return out

    if mode == "seq":
        return [c for blk in range(NBLK) for d in range(D + 1)
                for c in level(blk, d)]
    if mode in ("paired", "paired2"):
        # pair block b's leaves (ACT-heavy) with block b-1's internals
        # (PE-heavy) so per-iteration engine load stays balanced
        stream = level(0, 0)
        last = NBLK - 1
        for b in range(1, NBLK):
            internals = [c for d in range(1, D + 1) for c in level(b - 1, d)]
            if mode == "paired2" and b == last:
                # weave the final block's internals in as soon as their
                # children clear the pipeline, instead of a serial tail
                tail = [c for d in range(1, D + 1) for c in level(last, d)]
                leaves = level(last, 0)
                out = []
                done_leaf = 0
                ti = 0
                src = _interleave(leaves, internals)
                for c in src:
                    out.append(c)
                    if c[0] == last and c[1] == 0:
                        done_leaf += 1
                    # emit a tail chunk when its children are >=3 back
                    while ti < len(tail):
                        _, td, _, _, tc0, tmc = tail[ti]
                        if td != 1:
                            break
                        need = (2 * tc0 + 2 * tmc - 1) // MC + 1
                        if done_leaf >= need + 2:
                            out.append(tail[ti])
                            ti += 1
                        else:
                            break
                stream += out + tail[ti:]
                return stream
            stream += _interleave(level(b, 0), internals)
        stream += [c for d in range(1, D + 1) for c in level(last, d)]
        return stream
    if mode == "adaptive":
        # body: leaves+d1+d2 of all blocks, class-alternating and
        # readiness-aware; tail: d3..d5 of all blocks in lockstep rounds so
        # the four per-block chains hide each other's latency.
        body = {b: [c for d in range(3) for c in level(b, d)]
                for b in range(NBLK)}
        pos = {}
        stream = []

        def dep_ok(c):
            b, d, m0, sw, c0, mc = c
            if d == 0:
                return True
            hi = (2 * c0 + 2 * mc - 1) // MC
            p = pos.get((b, d - 1, hi))
            return p is not None and len(stream) >= p + 2

        def emit(c):
            pos[(c[0], c[1], c[4] // MC)] = len(stream)
            stream.append(c)

        last_leaf = False
        while any(body.values()):
            cands = [(b, body[b][0]) for b in range(NBLK)
                     if body[b] and dep_ok(body[b][0])]
            if not cands:
                stream.append(None)
                continue
            pref = [bc for bc in cands if (bc[1][1] == 0) != last_leaf]
            b, c = (pref or cands)[0]
            body[b].pop(0)
            emit(c)
            last_leaf = c[1] == 0
        for d in range(3, D + 1):
            for b in range(NBLK):
                for c in level(b, d):
                    emit(c)
        return stream
    if mode == "greedy":
        # readiness-aware class-alternating greedy: a chunk is ready when
        # the chunks producing its children are >= LAG positions back.
        LAG = 3
        todo = {(b, d): level(b, d) for b in range(NBLK) for d in range(D + 1)}
        done_pos = {}          # (b, d, chunk_idx) -> stream position
        emitted = {(b, d): 0 for b in range(NBLK) for d in range(D + 1)}
        stream = []
        debt = 0               # >0: too many leaves lately

        nch = {(b, d): len(level(b, d)) for b in range(NBLK)
               for d in range(D + 1)}

        def ready(b, d):
            lst = todo[(b, d)]
            if not lst:
                return False
            if emitted[(b, d)] == 0 and b >= 2:
                # hc pool safety: tile (b,d) reuses (b-2,d)'s buffer, whose
                # readers are (b-2,d+1)'s A stages and (b-2,d)'s own store
                if d < D and todo[(b - 2, d + 1)]:
                    return False
                p = done_pos.get((b - 2, d, nch[(b - 2, d)] - 1))
                if p is None or len(stream) - p < LAG:
                    return False
            if d == 0:
                return True
            _, _, _, _, c0, mc = lst[0]
            hi_child_chunk = (2 * c0 + 2 * mc - 1) // MC
            for ci in range(hi_child_chunk + 1):
                p = done_pos.get((b, d - 1, ci))
                if p is None or len(stream) - p < LAG:
                    return False
            return True

        total = sum(len(v) for v in todo.values())
        stall = 0
        while len(stream) < total:
            cands = [(b, d) for b in range(NBLK) for d in range(D + 1)
                     if ready(b, d)]
            if not cands:
                # force earliest unfinished level (will stall in hw a bit)
                cands = [min((k for k, v in todo.items() if v),
                             key=lambda k: (k[0], k[1]))]
            # prefer class that rebalances: leaves if debt<=0 else internal
            leaf_c = [c for c in cands if c[1] == 0]
            int_c = [c for c in cands if c[1] > 0]
            if debt <= 0 and leaf_c:
                pick = min(leaf_c)
            elif int_c:
                pick = min(int_c, key=lambda k: (k[1] > 0, k[0], k[1]))
            else:
                pick = min(cands)
            b, d = pick
            ck = todo[(b, d)].pop(0)
            idx = emitted[(b, d)]
            emitted[(b, d)] += 1
            done_pos[(b, d, idx)] = len(stream)
            stream.append(ck)
            debt += 1 if d == 0 else -1
        return stream
    raise ValueError(mode)


def build_nc(ep_bufs=4, ty_bufs=4, wk_bufs=3, sio_bufs=5, pf_bufs=1,
             eng=None, sub=1024, prefetch=4, mode="adaptive", hcs_bufs=8,
             hc0_bufs=2, hc1_bufs=2, hc2_bufs=4, ep_bufs2=None,
             cmerge=False):
    if eng is None:
        eng = {}
    # engine per op: 'v' = DVE (vector), 'g' = Pool (gpsimd)
    E_ = {
        "hty": "v", "cty": "v",            # ty-masked products [2mc]
        "ht1": "g", "ct1": "v",            # pairsums -> [mc]
        "hsum": "g", "csum": "g",
        "p1": "v", "th": "v", "sd": "v", "p2": "v", "p3": "v",
        "q": "v", "cc": "v", "hm": "v",
    }
    E_.update(eng)

    nc = bass.Bass()
    embt_d = nc.declare_dram_parameter(
        "embt", [128, 2, 2, CORE_COLS], mybir.dt.float8e4, isOutput=False)
    ty_d = nc.declare_dram_parameter(
        "tyrep", [128, TY_TOTAL], mybir.dt.bfloat16, isOutput=False)
    wf8_d = nc.declare_dram_parameter(
        "wf8", [128, 8, 2, 128], mybir.dt.float8e4, isOutput=False)
    wb_d = nc.declare_dram_parameter(
        "wb", [128, 10, 128], mybir.dt.bfloat16, isOutput=False)
    hout_d = nc.declare_dram_parameter(
        "hout", [128, CORE_COLS], mybir.dt.bfloat16, isOutput=True)

    # ---- flat chunk / sub schedules (sub ids in stream order).
    # Insert None bubbles so a chunk sits >= 2 stream positions after the
    # last producer chunk its child reads depend on (emission order per
    # iteration is C,B,A, so distance 2 guarantees read-after-write).
    raw = _chunk_stream(mode, sub)
    subs = []
    chunks = []
    sub_ids = {}
    seen_levels = set()
    cpos = {}
    for blk, d, m0, sw, c0, mc in raw:
        if d > 0:
            hi_child = (2 * c0 + 2 * mc - 1) // MC
            p = cpos[(blk, d - 1, hi_child)]
            while len(chunks) < p + 2:
                chunks.append(None)
        off = _lvl_off(blk, d)
        toff = _ty_off(blk, d)
        skey = (blk, d, m0)
        new_sub = skey not in sub_ids
        if new_sub:
            sub_ids[skey] = len(subs)
            subs.append(dict(blk=blk, d=d, off=off, toff=toff, m0=m0, sw=sw))
        new_level = (blk, d) not in seen_levels
        seen_levels.add((blk, d))
        cpos[(blk, d, c0 // MC)] = len(chunks)
        chunks.append(dict(
            blk=blk, d=d, off=off, m0=m0, sw=sw, c0=c0, mc=mc,
            sid=sub_ids[skey], new_sub=new_sub,
            last_in_sub=(c0 + mc == m0 + sw),
            last_of_level=(c0 + mc == LVL_M[d]),
            new_level=new_level))

    with tile.TileContext(nc) as tc, \
            tc.tile_pool(name="consts", bufs=1) as consts, \
            tc.tile_pool(name="emb", bufs=ep_bufs) as ep, \
            tc.tile_pool(name="ty", bufs=ty_bufs) as typ, \
            tc.tile_pool(name="hc0", bufs=hc0_bufs) as hc0pool, \
            tc.tile_pool(name="hc1", bufs=hc1_bufs) as hc1pool, \
            tc.tile_pool(name="hc2", bufs=hc2_bufs) as hc2pool, \
            tc.tile_pool(name="hcs", bufs=hcs_bufs) as hcspool, \
            tc.tile_pool(name="sio", bufs=sio_bufs) as siop, \
            tc.tile_pool(name="work", bufs=wk_bufs) as wk, \
            tc.tile_pool(name="psA", bufs=2, space="PSUM") as psA, \
            tc.tile_pool(name="psB", bufs=pf_bufs, space="PSUM") as psB:

        wf8_t = consts.tile([128, 8, 2, 128], mybir.dt.float8e4)
        nc.sync.dma_start(out=wf8_t, in_=wf8_d[:, :, :, :])
        wb_t = consts.tile([128, 10, 128], mybir.dt.bfloat16)
        nc.sync.dma_start(out=wb_t, in_=wb_d[:, :, :])

        def W8(s):
            return wf8_t[:, s, :, :]

        def WB(s):
            return wb_t[:, s, :]

        mm = nc.tensor.matmul
        act = nc.scalar.activation

        def V(which):
            return nc.vector if E_[which] == "v" else nc.gpsimd

        emb_tiles = {}
        ty_tiles = {}
        hc_tiles = {}
        next_load = [0]

        def load_sub(j):
            if j >= len(subs):
                return
            sb = subs[j]
            et = ep.tile([128, 2, 2, sb["sw"]], mybir.dt.float8e4, tag="emb")
            a = sb["off"] + sb["m0"]
            nc.sync.dma_start(out=et, in_=embt_d[:, :, :, a:a + sb["sw"]])
            emb_tiles[j] = et
            if sb["d"] > 0:
                tt = typ.tile([128, 2 * sb["sw"]], mybir.dt.bfloat16, tag="ty")
                a = sb["toff"] + 2 * sb["m0"]
                nc.sync.dma_start(out=tt, in_=ty_d[:, a:a + 2 * sb["sw"]])
                ty_tiles[j] = tt

        def stage_a(ck):
            """Child reductions + emb-side fp8 DoubleRow matmuls."""
            blk, d, c0, mc = ck["blk"], ck["d"], ck["c0"], ck["mc"]
            if ck["new_sub"]:
                while next_load[0] <= ck["sid"] + prefetch:
                    load_sub(next_load[0])
                    next_load[0] += 1
            if ck["new_level"]:
                pool, tg = {
                    0: (hc0pool, "hc0"), 1: (hc1pool, "hc1"),
                    2: (hc2pool, "hc2"),
                }.get(d, (hcspool, "hcs"))
                hc_tiles[(blk, d)] = pool.tile(
                    [128, 2, LVL_M[d]], mybir.dt.bfloat16, tag=tg, name="hc")
            st = dict(ck)
            st["hc_cur"] = hc_tiles[(blk, d)]
            embt = emb_tiles[ck["sid"]]
            st["xhi"] = embt[:, 0, :, c0 - ck["m0"]:c0 - ck["m0"] + mc]
            st["xlo"] = embt[:, 1, :, c0 - ck["m0"]:c0 - ck["m0"] + mc]

            if d > 0:
                hc_prev = hc_tiles[(blk, d - 1)]
                hcp = hc_prev[:, :, 2 * c0:2 * c0 + 2 * mc]
                tyt = ty_tiles[ck["sid"]]
                tyv = tyt[:, 2 * (c0 - ck["m0"]):2 * (c0 - ck["m0"]) + 2 * mc]
                hcty = wk.tile([128, 2, 2 * mc], mybir.dt.bfloat16, tag="hcty")
                V("hty").tensor_mul(hcty[:, 0, :], hcp[:, 0, :], tyv)
                V("cty").tensor_mul(hcty[:, 1, :], hcp[:, 1, :], tyv)
                h3 = hcty.rearrange("p t (m two) -> p t m two", two=2)
                hp3 = hcp.rearrange("p t (m two) -> p t m two", two=2)
                hct1 = wk.tile([128, 2, mc], mybir.dt.bfloat16, tag="hct1")
                V("ht1").tensor_add(hct1[:, 0, :], h3[:, 0, :, 0], h3[:, 0, :, 1])
                V("ct1").tensor_add(hct1[:, 1, :], h3[:, 1, :, 0], h3[:, 1, :, 1])
                hcsum = wk.tile([128, 2, mc], mybir.dt.bfloat16, tag="hcsum")
                V("hsum").tensor_add(hcsum[:, 0, :], hp3[:, 0, :, 0],
                                     hp3[:, 0, :, 1])
                V("csum").tensor_add(hcsum[:, 1, :], hp3[:, 1, :, 0],
                                     hp3[:, 1, :, 1])
                st["hct1"], st["hcsum"] = hct1, hcsum

            piou = psA.tile([128, 3, MC], mybir.dt.float32, tag="iou")
            for g in range(3):
                o_ = piou[:, g, :mc]
                mm(o_, W8(2 * g), st["xhi"], start=True, stop=False,
                   perf_mode=DR)
                mm(o_, W8(2 * g), st["xlo"], start=False, stop=False,
                   perf_mode=DR)
                mm(o_, W8(2 * g + 1), st["xhi"], start=False, stop=(d == 0),
                   perf_mode=DR)
            st["piou"] = piou
            return st

        def stage_b(st):
            """h-side matmuls, f matmuls, sigmoid acts, c assembly."""
            d, c0, mc = st["d"], st["c0"], st["mc"]
            piou = st["piou"]
            if d > 0:
                hsum = st["hcsum"][:, 0, :]
                ht1 = st["hct1"][:, 0, :]
                csum = st["hcsum"][:, 1, :]
                ct1 = st["hct1"][:, 1, :]
                for g in range(3):
                    o_ = piou[:, g, :mc]
                    mm(o_, WB(2 * g), hsum, start=False, stop=False)
                    mm(o_, WB(2 * g + 1), ht1, start=False, stop=True)
                pf = psB.tile([128, 2, MC], mybir.dt.float32, tag="f")
                for fi in range(2):
                    o_ = pf[:, fi, :mc]
                    mm(o_, W8(6), st["xhi"], start=True, stop=False,
                       perf_mode=DR)
                    mm(o_, W8(6), st["xlo"], start=False, stop=False,
                       perf_mode=DR)
                    mm(o_, W8(7), st["xhi"], start=False, stop=False,
                       perf_mode=DR)
                mm(pf[:, 0, :mc], WB(6), hsum, start=False, stop=False)
                mm(pf[:, 0, :mc], WB(7), ht1, start=False, stop=True)
                mm(pf[:, 1, :mc], WB(8), hsum, start=False, stop=False)
                mm(pf[:, 1, :mc], WB(9), ht1, start=False, stop=True)

            sio = siop.tile([128, 3, mc], mybir.dt.bfloat16, tag="sio")
            act(sio, piou[:, :, :mc], SIG, scale=1.0 / WSC)
            si, so, su = sio[:, 0, :], sio[:, 1, :], sio[:, 2, :]
            st["so"] = so

            th = wk.tile([128, mc], mybir.dt.bfloat16, tag="th")
            V("th").tensor_scalar(th, su, 2.0, 1.0, op0=MUL, op1=SUBT)
            ccs = st["hc_cur"][:, 1, c0:c0 + mc]
            if d == 0:
                V("p1").tensor_mul(ccs, si, th)
            else:
                sf = siop.tile([128, 2, mc], mybir.dt.bfloat16, tag="sf")
                act(sf, pf[:, :, :mc], SIG, scale=1.0 / WSC)
                s0, s1 = sf[:, 0, :], sf[:, 1, :]
                p1 = wk.tile([128, mc], mybir.dt.bfloat16, tag="p1")
                V("p1").tensor_mul(p1, si, th)
                sd = wk.tile([128, mc], mybir.dt.bfloat16, tag="sd")
                V("sd").tensor_sub(sd, s1, s0)
                p2 = wk.tile([128, mc], mybir.dt.bfloat16, tag="p2")
                V("p2").tensor_mul(p2, s0, csum)
                p3 = wk.tile([128, mc], mybir.dt.bfloat16, tag="p3")
                V("p3").tensor_mul(p3, sd, ct1)
                q = wk.tile([128, mc], mybir.dt.bfloat16, tag="q")
                V("q").tensor_add(q, p1, p2)
                V("cc").tensor_add(ccs, q, p3)
            st["ccs"] = ccs
            return st

        def _store_sub(st):
            if st["last_in_sub"]:
                m0, sw = st["m0"], st["sw"]
                nc.sync.dma_start(
                    out=hout_d[:, st["off"] + m0:st["off"] + m0 + sw],
                    in_=st["hc_cur"][:, 0, m0:m0 + sw])

        def flush_c(st):
            """tanh(c), h = so*tanh(c), per-sub store."""
            c0, mc = st["c0"], st["mc"]
            tcv = wk.tile([128, mc], mybir.dt.bfloat16, tag="tc")
            act(tcv, st["ccs"], TANH)
            V("hm").tensor_mul(st["hc_cur"][:, 0, c0:c0 + mc], st["so"], tcv)
            _store_sub(st)

        pend_c = {}

        def stage_c(st):
            if not cmerge:
                flush_c(st)
                return
            key = (st["blk"], st["d"])
            held = pend_c.pop(key, None)
            if held is not None and held["c0"] + held["mc"] == st["c0"]:
                mch = held["mc"] + st["mc"]
                c0h = held["c0"]
                tcv = wk.tile([128, mch], mybir.dt.bfloat16, tag="tc")
                act(tcv, held["hc_cur"][:, 1, c0h:c0h + mch], TANH)
                V("hm").tensor_mul(
                    held["hc_cur"][:, 0, c0h:c0h + held["mc"]],
                    held["so"], tcv[:, :held["mc"]])
                V("hm").tensor_mul(
                    st["hc_cur"][:, 0, st["c0"]:st["c0"] + st["mc"]],
                    st["so"], tcv[:, held["mc"]:])
                _store_sub(held)
                _store_sub(st)
            elif st["last_of_level"]:
                if held is not None:
                    flush_c(held)
                flush_c(st)
            else:
                if held is not None:
                    flush_c(held)
                pend_c[key] = st

        def tagged(fn, st, lbl):
            if TRACE_TAGS is None:
                fn(st)
                return
            lo = nc.next_id()
            fn(st)
            hi = nc.next_id()
            TRACE_TAGS.append(
                (f"{lbl}:{st['blk']},{st['d']},{st['c0']}", lo, hi))

        # Emission order per iteration is C(k-2), B(k-1), A(k): a consumer
        # chunk's child reads (A) are only emitted once the producer chunk's
        # h write (C) is out, provided the stream keeps dependent chunks >= 2
        # positions apart (the stream builder inserts None bubbles for that).
        p1s = p2s = None
        for ck in chunks + [None, None]:
            if p2s is not None:
                tagged(stage_c, p2s, "C")
            nxt = None
            if ck is not None:
                if TRACE_TAGS is None:
                    nxt = stage_a(ck)
                else:
                    lo = nc.next_id()
                    nxt = stage_a(ck)
                    hi = nc.next_id()
                    TRACE_TAGS.append(
                        (f"A:{ck['blk']},{ck['d']},{ck['c0']}", lo, hi))
            if p1s is not None:
                tagged(stage_b, p1s, "B")
            p2s, p1s = p1s, nxt
        for st in list(pend_c.values()):
            flush_c(st)
    split_waits(nc)
    return nc


# ---------------- host side ----------------

def _col_perm():
    """perm0[col] -> node index within a core's tree-range (0..TPC*S)."""
    cols = []
    for blk in range(NBLK):
        for d in range(D + 1):
            for t in range(BT):
                tree = blk * BT + t
                base = tree * S + OFFS[d]
                cols.append(np.arange(base, base + COUNTS[d]))
    return np.concatenate(cols)


_NC_CACHE = {}


def _get_nc():
    if "nc" not in _NC_CACHE:
        _NC_CACHE["nc"] = build_nc()
    return _NC_CACHE["nc"]


def _fp8_hilo(x):
    hi = np.asarray(x, F32).astype(FP8)
    lo = (np.asarray(x, F32) - hi.astype(F32)).astype(FP8)
    return hi, lo


def prep_in_maps(emb, child_mask, W_iou, U_iou, b_iou, W_f, U_f_w, U_f_b, b_f,
                 children_idx, child_type):
    emb = np.asarray(emb, F32)
    W_iou = np.asarray(W_iou, F32)
    U_iou = np.asarray(U_iou, F32)
    W_f = np.asarray(W_f, F32)
    U_f_w = np.asarray(U_f_w, F32)
    child_type = np.asarray(child_type, np.int32)
    assert not np.any(np.asarray(b_iou, F32)) and \
        not np.any(np.asarray(U_f_b, F32)) and not np.any(np.asarray(b_f, F32))

    perm0 = _col_perm()

    # wf8 [128, 8, 2, 128]: hi/lo fp8 of WSC * W (u gate doubled)
    wf8 = np.zeros((128, 8, 2, 128), FP8)
    for g in range(4):
        if g < 3:
            Wg = W_iou[:, 128 * g:128 * (g + 1)] * (WSC * (2.0 if g == 2 else 1.0))
        else:
            Wg = W_f * WSC
        for k in range(2):
            hi, lo = _fp8_hilo(Wg[128 * k:128 * (k + 1), :])
            wf8[:, 2 * g, k, :] = hi
            wf8[:, 2 * g + 1, k, :] = lo

    # wb [128, 10, 128]: bf16 U-side, * WSC (u' doubled)
    wb = np.zeros((128, 10, 128), BF16)
    for g in range(3):
        sc = WSC * (2.0 if g == 2 else 1.0)
        U0 = U_iou[0:128, 128 * g:128 * (g + 1)] * sc
        U1 = U_iou[128:256, 128 * g:128 * (g + 1)] * sc
        wb[:, 2 * g, :] = U0.astype(BF16)
        wb[:, 2 * g + 1, :] = (U1 - U0).astype(BF16)
    A0 = U_f_w[0:128, 0:128] * WSC
    A1 = U_f_w[128:256, 0:128] * WSC
    B0 = U_f_w[0:128, 128:256] * WSC
    B1 = U_f_w[128:256, 128:256] * WSC
    wb[:, 6, :] = A0.astype(BF16)
    wb[:, 7, :] = (A1 - A0).astype(BF16)
    wb[:, 8, :] = B0.astype(BF16)
    wb[:, 9, :] = (B1 - B0).astype(BF16)

    emb3 = emb.reshape(NCORES, TPC * S, E)
    ct3 = child_type.reshape(NCORES, TPC, S, 2)

    in_maps = []
    for k in range(NCORES):
        emb_core = emb3[k][perm0]                      # [CORE_COLS, E]
        embT = np.ascontiguousarray(emb_core.T)        # [E, CORE_COLS]
        e4 = embT.reshape(2, 128, CORE_COLS)           # [k, 128, cols]
        hi, lo = _fp8_hilo(e4)
        embt = np.empty((128, 2, 2, CORE_COLS), FP8)
        embt[:, 0, :, :] = hi.transpose(1, 0, 2)
        embt[:, 1, :, :] = lo.transpose(1, 0, 2)

        typarts = []
        for blk in range(NBLK):
            for d in range(1, D + 1):
                sl = ct3[k, blk * BT:(blk + 1) * BT,
                         OFFS[d]:OFFS[d] + COUNTS[d], :]
                typarts.append(sl.reshape(-1))
        tyrow = np.concatenate(typarts).astype(BF16)
        tyrep = np.ascontiguousarray(
            np.broadcast_to(tyrow[None, :], (128, TY_TOTAL)))
        in_maps.append({
            "embt": embt, "tyrep": tyrep, "wf8": wf8, "wb": wb,
        })
    return in_maps


def kernel(**inputs):
    in_maps = prep_in_maps(**inputs)
    nc = _get_nc()
    res = run_bass_kernel_spmd(nc, in_maps, core_ids=list(range(NCORES)))
    global LAST_EXEC_NS
    LAST_EXEC_NS = res.exec_time_ns

    perm0 = _col_perm()
    h = np.empty((N, H), F32)
    h4 = h.reshape(NCORES, TPC * S, H)
    for k in range(NCORES):
        h4[k][perm0] = res.results[k]["hout"].T.astype(F32)
    return h


# revision 38
# speedup vs baseline: 1.0633x; 1.0633x over previous
"""DepTreeLSTM forward on 8 Trainium2 NeuronCores (Bass/Tile) — v3.

Forest of T=4096 full binary trees (depth 5, 63 nodes each), TreeLSTM swept
level-by-level (leaves -> root). Trees are data-parallel: 512 trees per core.

Channels-on-partitions layout: per-node work is column-parallel; within a
core, node columns are ordered (block, level, tree, pos) so the children of
parent column m at level d are columns 2m, 2m+1 of level d-1 (stride-2
access, no indirection).

Structure:
  - emb-side matmuls run as fp8(e4m3) hi/lo DoubleRow pairs: the host splits
    emb and the W weights into fp8 hi + lo planes; W@emb = Whi@xhi + Whi@xlo
    + Wlo@xhi, each a 256-deep DoubleRow matmul.  Weights are pre-scaled by
    16 (fp8 normal range); the 1/16 dequant is folded into the activation
    `scale`.  h-side (U) matmuls stay bf16 (also pre-scaled by 16).
  - all biases in this problem are zero, so sigmoid over [i|o|u'] is ONE
    activation instruction (u' = 2u via doubled u-weights; tanh(u) =
    2*sig(2u)-1 is fixed up in the c assembly), and sigmoid over [f0|f1] is
    one more.  tanh(c) is the third.
  - ty rows are pre-replicated to 128 partitions in DRAM (plain HWDGE load);
    h is stored to DRAM as bf16, one DMA per 2048-col sub-slab.
  - per level: ht1/ct1 = pairsum(ty*h / ty*c), hsum/csum = pairsum(h / c);
    c = si*(2*su-1) + s0*csum + (s1-s0)*ct1; h = so*tanh(c).
  - chunks flow through an explicit 3-stage software pipeline (A: child
    reductions + emb matmuls; B: h-side matmuls, sigmoid acts, c assembly;
    C: tanh, h multiply, store) so every engine's in-order sequencer always
    has ready work from an older chunk ahead of it.
"""

import math

import numpy as np
import ml_dtypes

import concourse.bass as bass
import concourse.tile as tile
from concourse import mybir
from concourse.bass_utils import run_bass_kernel_spmd

BF16 = ml_dtypes.bfloat16
FP8 = ml_dtypes.float8_e4m3
F32 = np.float32

# ---------------- problem constants (hardcoded) ----------------
T, C, D, E, H = 4096, 2, 5, 256, 128
COUNTS = [C ** (D - d) for d in range(D + 1)]      # [32,16,8,4,2,1]
OFFS = [0, 32, 48, 56, 60, 62]
S = 63
N = T * S
NCORES = 8
TPC = T // NCORES                                   # 512 trees / core
NBLK = 4                                            # tree blocks / core
BT = TPC // NBLK                                    # 128 trees / block
LVL_M = [BT * c for c in COUNTS]                    # [8192,4096,2048,1024,512,256]
BLK_COLS = BT * S                                   # 16128
CORE_COLS = TPC * S                                 # 32256
TY_BLK = 2 * sum(LVL_M[1:])                         # 15872 child slots / block
TY_TOTAL = NBLK * TY_BLK
MC = 512                                            # chunk (= PSUM bank)
SUB = 2048                                          # emb/ty staging sub-slab
WSC = 16.0                                          # fp8 weight pre-scale

SIG = mybir.ActivationFunctionType.Sigmoid
TANH = mybir.ActivationFunctionType.Tanh
MUL = mybir.AluOpType.mult
ADD = mybir.AluOpType.add
SUBT = mybir.AluOpType.subtract
DR = mybir.MatmulPerfMode.DoubleRow

LAST_EXEC_NS = None
TRACE_TAGS = None  # set to a list to record (label, id_lo, id_hi) per stage


def split_waits(nc, nop_max=1, keep_max=1):
    """Walrus in this container rejects instructions with too many sem-waits
    (Drain: 0 allowed, NoOp: 1, others: 2). Move excess waits onto inserted
    NoOps, one wait each."""
    n_fix = 0
    for f in nc.m.functions:
        for bb in f.blocks:
            insts = bb.instructions
            i = 0
            while i < len(insts):
                ins = insts[i]
                si = getattr(ins, "sync_info", None)
                ow = list(si.on_wait) if si and si.on_wait else []
                keep = 0 if type(ins).__name__ == "InstDrain" else keep_max
                if len(ow) > keep:
                    extra = ow[:len(ow) - keep]
                    si.on_wait = ow[len(ow) - keep:]
                    k = 0
                    while extra:
                        chunk, extra = extra[:nop_max], extra[nop_max:]
                        nop = mybir.InstNoOp(
                            name=f"I-wsplit-{ins.name}-{k}", engine=ins.engine,
                            ins=[], outs=[])
                        nop.sync_info = type(si)(on_wait=chunk, on_update=[])
                        insts.insert(i, nop)
                        i += 1
                        k += 1
                        n_fix += 1
                i += 1
    return n_fix


def _lvl_off(blk, d):
    return blk * BLK_COLS + sum(LVL_M[:d])


def _ty_off(blk, d):
    return blk * TY_BLK + 2 * sum(LVL_M[1:d])


# wf8 slots [128, 8, 2, 128]: per gate g in (i,o,u',f): slot 2g = (W0hi,W1hi),
# slot 2g+1 = (W0lo,W1lo).  u' slots hold 2*W_u.
# wb slots [128, 10, 128]: U0_i, U1d_i, U0_o, U1d_o, U0_u', U1d_u', A, Ad, B, Bd
# (U1d = U1 - U0; u' slots hold 2*U_u; all * WSC)


def _interleave(a, b):
    out = []
    ia = ib = 0
    while ia < len(a) or ib < len(b):
        if ia < len(a):
            out.append(a[ia])
            ia += 1
        if ib < len(b):
            out.append(b[ib])
            ib += 1
    return out


def _chunk_stream(mode, sub):
    """Ordered list of (blk, d, m0, sw, c0, mc) chunk descriptors."""
    def level(blk, d):
        out = []
        M = LVL_M[d]
        for m0 in range(0, M, sub):
            sw = min(sub, M - m0)
            for c0 in range(m0, m0 + sw, MC):
                out.append((blk, d, m0, sw, c0, min(MC, m0 + sw - c0)))
        return out

    if mode == "seq":
        return [c for blk in range(NBLK) for d in range(D + 1)
                for c in level(blk, d)]
    if mode in ("paired", "paired2"):
        # pair block b's leaves (ACT-heavy) with block b-1's internals
        # (PE-heavy) so per-iteration engine load stays balanced
        stream = level(0, 0)
        last = NBLK - 1
        for b in range(1, NBLK):
            internals = [c for d in range(1, D + 1) for c in level(b - 1, d)]
            if mode == "paired2" and b == last:
                # weave the final block's internals in as soon as their
                # children clear the pipeline, instead of a serial tail
                tail = [c for d in range(1, D + 1) for c in level(last, d)]
                leaves = level(last, 0)
                out = []
                done_leaf = 0
                ti = 0
                src = _interleave(leaves, internals)
                for c in src:
                    out.append(c)
                    if c[0] == last and c[1] == 0:
                        done_leaf += 1
                    # emit a tail chunk when its children are >=3 back
                    while ti < len(tail):
                        _, td, _, _, tc0, tmc = tail[ti]
                        if td != 1:
                            break
                        need = (2 * tc0 + 2 * tmc - 1) // MC + 1
                        if done_leaf >= need + 2:
                            out.append(tail[ti])
                            ti += 1
                        else:
                            break
                stream += out + tail[ti:]
                return stream
            stream += _interleave(level(b, 0), internals)
        stream += [c for d in range(1, D + 1) for c in level(last, d)]
        return stream
    if mode == "adaptive":
        # body: leaves+d1+d2 of all blocks, class-alternating and
        # readiness-aware; tail: d3..d5 of all blocks in lockstep rounds so
        # the four per-block chains hide each other's latency.
        body = {b: [c for d in range(3) for c in level(b, d)]
                for b in range(NBLK)}
        pos = {}
        stream = []

        def dep_ok(c):
            b, d, m0, sw, c0, mc = c
            if d == 0:
                return True
            hi = (2 * c0 + 2 * mc - 1) // MC
            p = pos.get((b, d - 1, hi))
            return p is not None and len(stream) >= p + 2

        def emit(c):
            pos[(c[0], c[1], c[4] // MC)] = len(stream)
            stream.append(c)

        last_leaf = False
        while any(body.values()):
            cands = [(b, body[b][0]) for b in range(NBLK)
                     if body[b] and dep_ok(body[b][0])]
            if not cands:
                stream.append(None)
                continue
            pref = [bc for bc in cands if (bc[1][1] == 0) != last_leaf]
            b, c = (pref or cands)[0]
            body[b].pop(0)
            emit(c)
            last_leaf = c[1] == 0
        for d in range(3, D + 1):
            for b in range(NBLK):
                for c in level(b, d):
                    emit(c)
        return stream
    if mode == "greedy":
        # readiness-aware class-alternating greedy: a chunk is ready when
        # the chunks producing its children are >= LAG positions back.
        LAG = 3
        todo = {(b, d): level(b, d) for b in range(NBLK) for d in range(D + 1)}
        done_pos = {}          # (b, d, chunk_idx) -> stream position
        emitted = {(b, d): 0 for b in range(NBLK) for d in range(D + 1)}
        stream = []
        debt = 0               # >0: too many leaves lately

        nch = {(b, d): len(level(b, d)) for b in range(NBLK)
               for d in range(D + 1)}

        def ready(b, d):
            lst = todo[(b, d)]
            if not lst:
                return False
            if emitted[(b, d)] == 0 and b >= 2:
                # hc pool safety: tile (b,d) reuses (b-2,d)'s buffer, whose
                # readers are (b-2,d+1)'s A stages and (b-2,d)'s own store
                if d < D and todo[(b - 2, d + 1)]:
                    return False
                p = done_pos.get((b - 2, d, nch[(b - 2, d)] - 1))
                if p is None or len(stream) - p < LAG:
                    return False
            if d == 0:
                return True
            _, _, _, _, c0, mc = lst[0]
            hi_child_chunk = (2 * c0 + 2 * mc - 1) // MC
            for ci in range(hi_child_chunk + 1):
                p = done_pos.get((b, d - 1, ci))
                if p is None or len(stream) - p < LAG:
                    return False
            return True

        total = sum(len(v) for v in todo.values())
        stall = 0
        while len(stream) < total:
            cands = [(b, d) for b in range(NBLK) for d in range(D + 1)
                     if ready(b, d)]
            if not cands:
                # force earliest unfinished level (will stall in hw a bit)
                cands = [min((k for k, v in todo.items() if v),
                             key=lambda k: (k[0], k[1]))]
            # prefer class that rebalances: leaves if debt<=0 else internal
            leaf_c = [c for c in cands if c[1] == 0]
            int_c = [c for c in cands if c[1] > 0]
            if debt <= 0 and leaf_c:
                pick = min(leaf_c)
            elif int_c:
                pick = min(int_c, key=lambda k: (k[1] > 0, k[0], k[1]))
            else:
                pick = min(cands)
            b, d = pick
            ck = todo[(b, d)].pop(0)
            idx = emitted[(b, d)]
            emitted[(b, d)] += 1
            done_pos[(b, d, idx)] = len(stream)
            stream.append(ck)
            debt += 1 if d == 0 else -1
        return stream
    raise ValueError(mode)


def build_nc(ep_bufs=8, ty_bufs=8, wk_bufs=3, sio_bufs=5, pf_bufs=1,
             eng=None, sub=512, prefetch=6, mode="adaptive", hcs_bufs=8,
             hc0_bufs=2, hc1_bufs=2, hc2_bufs=4, ep_bufs2=None,
             cmerge=True):
    if eng is None:
        eng = {}
    # engine per op: 'v' = DVE (vector), 'g' = Pool (gpsimd)
    E_ = {
        "hty": "v", "cty": "v",            # ty-masked products [2mc]
        "ht1": "g", "ct1": "g",            # pairsums -> [mc]
        "hsum": "v", "csum": "g",
        "p1": "v", "th": "v", "sd": "v", "p2": "v", "p3": "v",
        "q": "v", "cc": "v", "hm": "v",
    }
    E_.update(eng)

    nc = bass.Bass()
    embt_d = nc.declare_dram_parameter(
        "embt", [128, 2, 2, CORE_COLS], mybir.dt.float8e4, isOutput=False)
    ty_d = nc.declare_dram_parameter(
        "tyrep", [128, TY_TOTAL], mybir.dt.bfloat16, isOutput=False)
    wf8_d = nc.declare_dram_parameter(
        "wf8", [128, 8, 2, 128], mybir.dt.float8e4, isOutput=False)
    wb_d = nc.declare_dram_parameter(
        "wb", [128, 10, 128], mybir.dt.bfloat16, isOutput=False)
    hout_d = nc.declare_dram_parameter(
        "hout", [128, CORE_COLS], mybir.dt.bfloat16, isOutput=True)

    # ---- flat chunk / sub schedules (sub ids in stream order).
    # Insert None bubbles so a chunk sits >= 2 stream positions after the
    # last producer chunk its child reads depend on (emission order per
    # iteration is C,B,A, so distance 2 guarantees read-after-write).
    raw = _chunk_stream(mode, sub)
    subs = []
    chunks = []
    sub_ids = {}
    seen_levels = set()
    cpos = {}
    for blk, d, m0, sw, c0, mc in raw:
        if d > 0:
            hi_child = (2 * c0 + 2 * mc - 1) // MC
            p = cpos[(blk, d - 1, hi_child)]
            while len(chunks) < p + 2:
                chunks.append(None)
        off = _lvl_off(blk, d)
        toff = _ty_off(blk, d)
        skey = (blk, d, m0)
        new_sub = skey not in sub_ids
        if new_sub:
            sub_ids[skey] = len(subs)
            subs.append(dict(blk=blk, d=d, off=off, toff=toff, m0=m0, sw=sw))
        new_level = (blk, d) not in seen_levels
        seen_levels.add((blk, d))
        cpos[(blk, d, c0 // MC)] = len(chunks)
        chunks.append(dict(
            blk=blk, d=d, off=off, m0=m0, sw=sw, c0=c0, mc=mc,
            sid=sub_ids[skey], new_sub=new_sub,
            last_in_sub=(c0 + mc == m0 + sw),
            last_of_level=(c0 + mc == LVL_M[d]),
            new_level=new_level))

    with tile.TileContext(nc) as tc, \
            tc.tile_pool(name="consts", bufs=1) as consts, \
            tc.tile_pool(name="emb", bufs=ep_bufs) as ep, \
            tc.tile_pool(name="ty", bufs=ty_bufs) as typ, \
            tc.tile_pool(name="hc0", bufs=hc0_bufs) as hc0pool, \
            tc.tile_pool(name="hc1", bufs=hc1_bufs) as hc1pool, \
            tc.tile_pool(name="hc2", bufs=hc2_bufs) as hc2pool, \
            tc.tile_pool(name="hcs", bufs=hcs_bufs) as hcspool, \
            tc.tile_pool(name="sio", bufs=sio_bufs) as siop, \
            tc.tile_pool(name="work", bufs=wk_bufs) as wk, \
            tc.tile_pool(name="psA", bufs=2, space="PSUM") as psA, \
            tc.tile_pool(name="psB", bufs=pf_bufs, space="PSUM") as psB:

        wf8_t = consts.tile([128, 8, 2, 128], mybir.dt.float8e4)
        nc.sync.dma_start(out=wf8_t, in_=wf8_d[:, :, :, :])
        wb_t = consts.tile([128, 10, 128], mybir.dt.bfloat16)
        nc.sync.dma_start(out=wb_t, in_=wb_d[:, :, :])

        def W8(s):
            return wf8_t[:, s, :, :]

        def WB(s):
            return wb_t[:, s, :]

        mm = nc.tensor.matmul
        act = nc.scalar.activation

        def V(which):
            return nc.vector if E_[which] == "v" else nc.gpsimd

        emb_tiles = {}
        ty_tiles = {}
        hc_tiles = {}
        next_load = [0]

        def load_sub(j):
            if j >= len(subs):
                return
            sb = subs[j]
            et = ep.tile([128, 2, 2, sb["sw"]], mybir.dt.float8e4, tag="emb")
            a = sb["off"] + sb["m0"]
            nc.sync.dma_start(out=et, in_=embt_d[:, :, :, a:a + sb["sw"]])
            emb_tiles[j] = et
            if sb["d"] > 0:
                tt = typ.tile([128, 2 * sb["sw"]], mybir.dt.bfloat16, tag="ty")
                a = sb["toff"] + 2 * sb["m0"]
                nc.sync.dma_start(out=tt, in_=ty_d[:, a:a + 2 * sb["sw"]])
                ty_tiles[j] = tt

        def stage_a(ck):
            """Child reductions + emb-side fp8 DoubleRow matmuls."""
            blk, d, c0, mc = ck["blk"], ck["d"], ck["c0"], ck["mc"]
            if ck["new_sub"]:
                while next_load[0] <= ck["sid"] + prefetch:
                    load_sub(next_load[0])
                    next_load[0] += 1
            if ck["new_level"]:
                pool, tg = {
                    0: (hc0pool, "hc0"), 1: (hc1pool, "hc1"),
                    2: (hc2pool, "hc2"),
                }.get(d, (hcspool, "hcs"))
                hc_tiles[(blk, d)] = pool.tile(
                    [128, 2, LVL_M[d]], mybir.dt.bfloat16, tag=tg, name="hc")
            st = dict(ck)
            st["hc_cur"] = hc_tiles[(blk, d)]
            embt = emb_tiles[ck["sid"]]
            st["xhi"] = embt[:, 0, :, c0 - ck["m0"]:c0 - ck["m0"] + mc]
            st["xlo"] = embt[:, 1, :, c0 - ck["m0"]:c0 - ck["m0"] + mc]

            if d > 0:
                hc_prev = hc_tiles[(blk, d - 1)]
                hcp = hc_prev[:, :, 2 * c0:2 * c0 + 2 * mc]
                tyt = ty_tiles[ck["sid"]]
                tyv = tyt[:, 2 * (c0 - ck["m0"]):2 * (c0 - ck["m0"]) + 2 * mc]
                hcty = wk.tile([128, 2, 2 * mc], mybir.dt.bfloat16, tag="hcty")
                V("hty").tensor_mul(hcty[:, 0, :], hcp[:, 0, :], tyv)
                V("cty").tensor_mul(hcty[:, 1, :], hcp[:, 1, :], tyv)
                h3 = hcty.rearrange("p t (m two) -> p t m two", two=2)
                hp3 = hcp.rearrange("p t (m two) -> p t m two", two=2)
                hct1 = wk.tile([128, 2, mc], mybir.dt.bfloat16, tag="hct1")
                V("ht1").tensor_add(hct1[:, 0, :], h3[:, 0, :, 0], h3[:, 0, :, 1])
                V("ct1").tensor_add(hct1[:, 1, :], h3[:, 1, :, 0], h3[:, 1, :, 1])
                hcsum = wk.tile([128, 2, mc], mybir.dt.bfloat16, tag="hcsum")
                V("hsum").tensor_add(hcsum[:, 0, :], hp3[:, 0, :, 0],
                                     hp3[:, 0, :, 1])
                V("csum").tensor_add(hcsum[:, 1, :], hp3[:, 1, :, 0],
                                     hp3[:, 1, :, 1])
                st["hct1"], st["hcsum"] = hct1, hcsum

            piou = psA.tile([128, 3, MC], mybir.dt.float32, tag="iou")
            for g in range(3):
                o_ = piou[:, g, :mc]
                mm(o_, W8(2 * g), st["xhi"], start=True, stop=False,
                   perf_mode=DR)
                mm(o_, W8(2 * g), st["xlo"], start=False, stop=False,
                   perf_mode=DR)
                mm(o_, W8(2 * g + 1), st["xhi"], start=False, stop=(d == 0),
                   perf_mode=DR)
            st["piou"] = piou
            return st

        def stage_b(st):
            """h-side matmuls, f matmuls, sigmoid acts, c assembly."""
            d, c0, mc = st["d"], st["c0"], st["mc"]
            piou = st["piou"]
            if d > 0:
                hsum = st["hcsum"][:, 0, :]
                ht1 = st["hct1"][:, 0, :]
                csum = st["hcsum"][:, 1, :]
                ct1 = st["hct1"][:, 1, :]
                for g in range(3):
                    o_ = piou[:, g, :mc]
                    mm(o_, WB(2 * g), hsum, start=False, stop=False)
                    mm(o_, WB(2 * g + 1), ht1, start=False, stop=True)
                pf = psB.tile([128, 2, MC], mybir.dt.float32, tag="f")
                for fi in range(2):
                    o_ = pf[:, fi, :mc]
                    mm(o_, W8(6), st["xhi"], start=True, stop=False,
                       perf_mode=DR)
                    mm(o_, W8(6), st["xlo"], start=False, stop=False,
                       perf_mode=DR)
                    mm(o_, W8(7), st["xhi"], start=False, stop=False,
                       perf_mode=DR)
                mm(pf[:, 0, :mc], WB(6), hsum, start=False, stop=False)
                mm(pf[:, 0, :mc], WB(7), ht1, start=False, stop=True)
                mm(pf[:, 1, :mc], WB(8), hsum, start=False, stop=False)
                mm(pf[:, 1, :mc], WB(9), ht1, start=False, stop=True)

            sio = siop.tile([128, 3, mc], mybir.dt.bfloat16, tag="sio")
            act(sio, piou[:, :, :mc], SIG, scale=1.0 / WSC)
            si, so, su = sio[:, 0, :], sio[:, 1, :], sio[:, 2, :]
            st["so"] = so

            th = wk.tile([128, mc], mybir.dt.bfloat16, tag="th")
            V("th").tensor_scalar(th, su, 2.0, 1.0, op0=MUL, op1=SUBT)
            ccs = st["hc_cur"][:, 1, c0:c0 + mc]
            if d == 0:
                V("p1").tensor_mul(ccs, si, th)
            else:
                sf = siop.tile([128, 2, mc], mybir.dt.bfloat16, tag="sf")
                act(sf, pf[:, :, :mc], SIG, scale=1.0 / WSC)
                s0, s1 = sf[:, 0, :], sf[:, 1, :]
                p1 = wk.tile([128, mc], mybir.dt.bfloat16, tag="p1")
                V("p1").tensor_mul(p1, si, th)
                sd = wk.tile([128, mc], mybir.dt.bfloat16, tag="sd")
                V("sd").tensor_sub(sd, s1, s0)
                p2 = wk.tile([128, mc], mybir.dt.bfloat16, tag="p2")
                V("p2").tensor_mul(p2, s0, csum)
                p3 = wk.tile([128, mc], mybir.dt.bfloat16, tag="p3")
                V("p3").tensor_mul(p3, sd, ct1)
                q = wk.tile([128, mc], mybir.dt.bfloat16, tag="q")
                V("q").tensor_add(q, p1, p2)
                V("cc").tensor_add(ccs, q, p3)
            st["ccs"] = ccs
            return st

        def _store_sub(st):
            if st["last_in_sub"]:
                m0, sw = st["m0"], st["sw"]
                nc.sync.dma_start(
                    out=hout_d[:, st["off"] + m0:st["off"] + m0 + sw],
                    in_=st["hc_cur"][:, 0, m0:m0 + sw])

        def flush_c(st):
            """tanh(c), h = so*tanh(c), per-sub store."""
            c0, mc = st["c0"], st["mc"]
            tcv = wk.tile([128, mc], mybir.dt.bfloat16, tag="tc")
            act(tcv, st["ccs"], TANH)
            V("hm").tensor_mul(st["hc_cur"][:, 0, c0:c0 + mc], st["so"], tcv)
            _store_sub(st)

        pend_c = {}

        def stage_c(st):
            if not cmerge:
                flush_c(st)
                return
            key = (st["blk"], st["d"])
            held = pend_c.pop(key, None)
            if held is not None and held["c0"] + held["mc"] == st["c0"]:
                mch = held["mc"] + st["mc"]
                c0h = held["c0"]
                tcv = wk.tile([128, mch], mybir.dt.bfloat16, tag="tc")
                act(tcv, held["hc_cur"][:, 1, c0h:c0h + mch], TANH)
                V("hm").tensor_mul(
                    held["hc_cur"][:, 0, c0h:c0h + held["mc"]],
                    held["so"], tcv[:, :held["mc"]])
                V("hm").tensor_mul(
                    st["hc_cur"][:, 0, st["c0"]:st["c0"] + st["mc"]],
                    st["so"], tcv[:, held["mc"]:])
                _store_sub(held)
                _store_sub(st)
            elif st["last_of_level"]:
                if held is not None:
                    flush_c(held)
                flush_c(st)
            else:
                if held is not None:
                    flush_c(held)
                pend_c[key] = st

        def tagged(fn, st, lbl):
            if TRACE_TAGS is None:
                fn(st)
                return
            lo = nc.next_id()
            fn(st)
            hi = nc.next_id()
            TRACE_TAGS.append(
                (f"{lbl}:{st['blk']},{st['d']},{st['c0']}", lo, hi))

        # Emission order per iteration is C(k-2), B(k-1), A(k): a consumer
        # chunk's child reads (A) are only emitted once the producer chunk's
        # h write (C) is out, provided the stream keeps dependent chunks >= 2
        # positions apart (the stream builder inserts None bubbles for that).
        p1s = p2s = None
        for ck in chunks + [None, None]:
            if p2s is not None:
                tagged(stage_c, p2s, "C")
            nxt = None
            if ck is not None:
                if TRACE_TAGS is None:
                    nxt = stage_a(ck)
                else:
                    lo = nc.next_id()
                    nxt = stage_a(ck)
                    hi = nc.next_id()
                    TRACE_TAGS.append(
                        (f"A:{ck['blk']},{ck['d']},{ck['c0']}", lo, hi))
            if p1s is not None:
                tagged(stage_b, p1s, "B")
            p2s, p1s = p1s, nxt
        for st in list(pend_c.values()):
            flush_c(st)
    split_waits(nc)
    return nc


# ---------------- host side ----------------

def _col_perm():
    """perm0[col] -> node index within a core's tree-range (0..TPC*S)."""
    cols = []
    for blk in range(NBLK):
        for d in range(D + 1):
            for t in range(BT):
                tree = blk * BT + t
                base = tree * S + OFFS[d]
                cols.append(np.arange(base, base + COUNTS[d]))
    return np.concatenate(cols)


_NC_CACHE = {}


def _get_nc():
    if "nc" not in _NC_CACHE:
        _NC_CACHE["nc"] = build_nc()
    return _NC_CACHE["nc"]


def _fp8_hilo(x):
    hi = np.asarray(x, F32).astype(FP8)
    lo = (np.asarray(x, F32) - hi.astype(F32)).astype(FP8)
    return hi, lo


def prep_in_maps(emb, child_mask, W_iou, U_iou, b_iou, W_f, U_f_w, U_f_b, b_f,
                 children_idx, child_type):
    emb = np.asarray(emb, F32)
    W_iou = np.asarray(W_iou, F32)
    U_iou = np.asarray(U_iou, F32)
    W_f = np.asarray(W_f, F32)
    U_f_w = np.asarray(U_f_w, F32)
    child_type = np.asarray(child_type, np.int32)
    assert not np.any(np.asarray(b_iou, F32)) and \
        not np.any(np.asarray(U_f_b, F32)) and not np.any(np.asarray(b_f, F32))

    perm0 = _col_perm()

    # wf8 [128, 8, 2, 128]: hi/lo fp8 of WSC * W (u gate doubled)
    wf8 = np.zeros((128, 8, 2, 128), FP8)
    for g in range(4):
        if g < 3:
            Wg = W_iou[:, 128 * g:128 * (g + 1)] * (WSC * (2.0 if g == 2 else 1.0))
        else:
            Wg = W_f * WSC
        for k in range(2):
            hi, lo = _fp8_hilo(Wg[128 * k:128 * (k + 1), :])
            wf8[:, 2 * g, k, :] = hi
            wf8[:, 2 * g + 1, k, :] = lo

    # wb [128, 10, 128]: bf16 U-side, * WSC (u' doubled)
    wb = np.zeros((128, 10, 128), BF16)
    for g in range(3):
        sc = WSC * (2.0 if g == 2 else 1.0)
        U0 = U_iou[0:128, 128 * g:128 * (g + 1)] * sc
        U1 = U_iou[128:256, 128 * g:128 * (g + 1)] * sc
        wb[:, 2 * g, :] = U0.astype(BF16)
        wb[:, 2 * g + 1, :] = (U1 - U0).astype(BF16)
    A0 = U_f_w[0:128, 0:128] * WSC
    A1 = U_f_w[128:256, 0:128] * WSC
    B0 = U_f_w[0:128, 128:256] * WSC
    B1 = U_f_w[128:256, 128:256] * WSC
    wb[:, 6, :] = A0.astype(BF16)
    wb[:, 7, :] = (A1 - A0).astype(BF16)
    wb[:, 8, :] = B0.astype(BF16)
    wb[:, 9, :] = (B1 - B0).astype(BF16)

    emb3 = emb.reshape(NCORES, TPC * S, E)
    ct3 = child_type.reshape(NCORES, TPC, S, 2)

    in_maps = []
    for k in range(NCORES):
        emb_core = emb3[k][perm0]                      # [CORE_COLS, E]
        embT = np.ascontiguousarray(emb_core.T)        # [E, CORE_COLS]
        e4 = embT.reshape(2, 128, CORE_COLS)           # [k, 128, cols]
        hi, lo = _fp8_hilo(e4)
        embt = np.empty((128, 2, 2, CORE_COLS), FP8)
        embt[:, 0, :, :] = hi.transpose(1, 0, 2)
        embt[:, 1, :, :] = lo.transpose(1, 0, 2)

        typarts = []
        for blk in range(NBLK):
            for d in range(1, D + 1):
                sl = ct3[k, blk * BT:(blk + 1) * BT,
                         OFFS[d]:OFFS[d] + COUNTS[d], :]
                typarts.append(sl.reshape(-1))
        tyrow = np.concatenate(typarts).astype(BF16)
        tyrep = np.ascontiguousarray(
            np.broadcast_to(tyrow[None, :], (128, TY_TOTAL)))
        in_maps.append({
            "embt": embt, "tyrep": tyrep, "wf8": wf8, "wb": wb,
        })
    return in_maps


def kernel(**inputs):
    in_maps = prep_in_maps(**inputs)
    nc = _get_nc()
    res = run_bass_kernel_spmd(nc, in_maps, core_ids=list(range(NCORES)))
    global LAST_EXEC_NS
    LAST_EXEC_NS = res.exec_time_ns

    perm0 = _col_perm()
    h = np.empty((N, H), F32)
    h4 = h.reshape(NCORES, TPC * S, H)
    for k in range(NCORES):
        h4[k][perm0] = res.results[k]["hout"].T.astype(F32)
    return h
